# revision 22
# baseline (speedup 1.0000x reference)
"""Trainium2 Bass kernel for nn_CMPModel (complex density matrix).

Math (per batch b, S=128 tokens, D=256):
    R = word_emb[questions[b]]                # [S, D]
    I = cmp_emb[questions[b]] * pos[b][:, None]
    real = R^T W R + I^T W I                  # symmetric   (W = diag(weighted_q))
    imag = I^T W R - R^T W I                  # antisymmetric

We compute only C = real + imag on device. Since diag weights can migrate
to either matmul operand (they depend only on the contraction index s),
two PSUM-accumulated products with 3 prepped operand tiles:
    C = A^T r + B^T wposc
      wposc = (w*pos)*c
      A     = w*r + wposc
      B     = pos*c - r
check (per element, sum over s):
    A^T r       = R^T W R + I^T W R
    B^T wposc   = sum_s (pos*c - r)*(w*pos*c) = I^T W I - R^T W I
and recover on host during unshard (exact by symmetry):
    real = (C + C^T)/2,  imag = (C - C^T)/2.

Sharding: data-parallel over batch, 8 batches per core, embedding tables
replicated. Gather via gpsimd indirect DMA against a host-interleaved
[V, 2D] table (row q = word_emb[q] ++ cmp_emb[q]), one [P,1]-offset
gather per batch. Multi-offset indirect DMA was probed on HW and is
broken (the ucode does not read the offset values from the q2 tile at
all; source addresses come out linear in the dest run index with a
constant sub-row misalignment), so one gather instruction per batch is
a hard floor: the Pool engine's serial descriptor generation
(~994ns fixed + 0.34ns/desc per instruction) paces the pipeline at
~1.4us/batch.

Engine balance per batch (each stage must stay under the ~1.4us gather
pace): ACT does wposc (activation Copy with per-partition scale) and
half the PSUM->SBUF copies; DVE does the two scalar_tensor_tensor preps
and the other half of the copies; PE does 4 LDWEIGHTS+MATMUL pairs
(bf16: ~420ns/MM with LDW overlapped). All compute rides inside the
gather window; the tail after the last gather is
data(+0.8) -> wposc -> a_t -> 4 MMs -> half-copies -> out-DMAs split
across the sync and ACT HWDGE sequencers (issue is ~0.65us serial per
instruction per engine).

Measured HW landmarks (fast-clock runs; DVFS adds ~10% run-to-run):
start rendezvous ~3.2us, TPBBaseLd ~1.2us, q2-on-sync done ~7.2us,
first gather 9.0us, gather pace 1.43us, last out-DMA issued 26.4us,
exec ~30.9us (vs 40.4us baseline). Alternatives probed and rejected on
HW: multi-offset indirect DMA (ucode broken, ignores offset values),
dma_gather ucode (2.7us/inst + 8us library load), pre-barrier q2 load
(holds the tile entry barrier), mid-pipeline ACT out-DMAs (blows ACT's
per-batch budget).
"""

import ml_dtypes
import numpy as np

import concourse.bacc as bacc
import concourse.bass as bass
import concourse.mybir as mybir
import concourse.tile as tile
from concourse.bass_utils import run_bass_kernel_spmd

V, D, S, B = 50000, 256, 128, 64
NCORES = 8
NB = B // NCORES          # batches per core
P = 128
F32 = mybir.dt.float32
# matmul input dtype. bf16 showed no speedup in earlier sessions because
# the span was gather-bound and PE hid underneath; after the pipeline
# fixes PE surfaced as the tail pacer (~1.7us/batch f32r: 4 LDW+MM pairs
# at ~240ns fixed overhead each), so bf16's faster weight loads + 2x DVE
# rate now matter. rel err ~3.5e-3 vs the 2e-2 gate.
USE_BF16 = True
F32R = mybir.dt.bfloat16 if USE_BF16 else mybir.dt.float32r
TBL_NP = ml_dtypes.bfloat16 if USE_BF16 else np.float32
MUL = mybir.AluOpType.mult
ADD = mybir.AluOpType.add
SUB = mybir.AluOpType.subtract

# set by test harness: trace the run and stash exec_time_ns
TRACE = False
LAST_EXEC_NS = None
LAST_RESULTS = None


def build_bass():
    nc = bacc.Bacc("TRN2", enable_partition_id=False)
    tables = nc.declare_dram_parameter("tables", [V, 2 * D], F32R, isOutput=False)
    q2_d = nc.declare_dram_parameter("q2", [P, NB], mybir.dt.int32, isOutput=False)
    # pos and wq packed into ONE upload: wpos (the first ACT op) then has a
    # single DMA-sem wait, so no EventSemaphore split lands before the
    # auto-inserted ACT_TABLE_LOAD — measured to pull the table load from
    # t=10.7us back to t=6.8us (it otherwise blocks every wposc).
    pk_d = nc.declare_dram_parameter("pk", [P, NB + 1], F32, isOutput=False)
    out_d = nc.declare_dram_parameter("outc", [NB, P, 2, D], F32, isOutput=True)

    # (A pre-barrier q2 load via raw tensor + manual sem was tried and
    # measured WORSE by ~1us: gpsimd then holds the tile entry barrier
    # until q2 lands, so the barrier cost moves after the DMA instead of
    # overlapping it. Keep q2 as the first in-context gpsimd DMA.)
    with tile.TileContext(nc) as tc:
        with (
            tc.tile_pool(name="const", bufs=1) as constp,
            tc.tile_pool(name="gather", bufs=NB) as gatherp,
            tc.tile_pool(name="work", bufs=3 * NB) as workp,
            tc.tile_pool(name="outp", bufs=8) as outp,
            tc.tile_pool(name="psum", bufs=8, space="PSUM") as psump,
        ):
            # q2 on sync, FIRST: sync's post-barrier DMA dispatch starts
            # ~0.7us before gpsimd's would, and gpsimd then enters the
            # gather stream directly (its first instruction waits the q2
            # DMA sem). pk second on sync; wpos isn't needed until the
            # first gather's data lands, so its extra latency is free.
            q2 = constp.tile([P, NB], mybir.dt.int32)
            pk = constp.tile([P, NB + 1], F32)
            nc.sync.dma_start(out=q2[:], in_=q2_d[:])
            nc.sync.dma_start(out=pk[:], in_=pk_d[:])
            # wpos on ACT: ACT is otherwise idle here, and making this the
            # first activation hoists the ACT_TABLE_LOAD to the program
            # start (it otherwise lands behind the first gather's sem wait,
            # delaying every wposc).
            wpos = constp.tile([P, NB], F32)
            nc.scalar.mul(wpos[:], pk[:, 0:NB], pk[:, NB : NB + 1])

            # all gathers up front: gpsimd desc-gen is the serial resource;
            # per-batch tiles with NB bufs so the stream never stalls on slots
            rcs = []
            for b in range(NB):
                rc = gatherp.tile([P, 2 * D], F32R, tag=f"rc")
                rcs.append(rc)
                nc.gpsimd.indirect_dma_start(
                    out=rc[:],
                    out_offset=None,
                    in_=tables[:],
                    in_offset=bass.IndirectOffsetOnAxis(ap=q2[:, b : b + 1], axis=0),
                )

            # per-batch pipeline: each batch's prep only needs its own gather,
            # so compute trails the gather stream by ~1 batch.
            # PSUM evacuation is interleaved with a 2-batch lag: emitting all
            # copies after all preps (tried) serializes every copy behind the
            # LAST prep in the in-order engine streams — batch 0's copy then
            # runs at T+26us with data ready at T+19us, and the out-DMA queue
            # backs up at the tail. With lag 2, copy b-2 waits on matmuls
            # that are already done when prep b is emitted, so neither stream
            # stalls. Even batches copy on DVE, odd on ACT; out-DMA issue
            # (~0.65us serial per instruction) all on the sync sequencer,
            # spread across the pipeline instead of bunched at the end.
            LAG = 2

            def emit_drain(b):
                # Tail DMAs (b5, b6, last half) issue on ACT: out-DMA issue
                # is ~0.65us serial per instruction, and sync alone bunches
                # the last 5 issues after the copies. ACT is done with
                # wposc/copies by then; mid-pipeline ACT DMAs (tried) blow
                # its per-batch budget and delay the wposc stream.
                out_sb = outp.tile([P, 2, D], F32, tag="osb")
                if b < NB - 1:
                    if b % 2 == 0:
                        nc.vector.tensor_copy(out_sb[:], pss[b][:])
                    else:
                        nc.scalar.copy(out_sb[:], pss[b][:])
                    # ACT only drains b5 and the final half-tile: giving it
                    # b6's DMA too (tried) queues the last half-copy behind
                    # that issue and delays the final DMA by ~0.7us
                    eng = nc.scalar if b == NB - 3 else nc.sync
                    eng.dma_start(out=out_d[b], in_=out_sb[:])
                else:
                    # last batch drains in halves so its DMA overlaps its copy
                    nc.vector.tensor_copy(out_sb[:, 0, :], pss[b][:, 0, :])
                    nc.sync.dma_start(out=out_d[b, :, 0], in_=out_sb[:, 0, :])
                    nc.scalar.copy(out_sb[:, 1, :], pss[b][:, 1, :])
                    nc.scalar.dma_start(out=out_d[b, :, 1], in_=out_sb[:, 1, :])

            pss = []
            wposcs = []
            for b in range(NB):
                rc = rcs[b]
                r_b = rc[:, 0:D]
                c_b = rc[:, D : 2 * D]
                wposc = workp.tile([P, D], F32R, tag="wposc")
                wposcs.append(wposc)
                a_t = workp.tile([P, D], F32R, tag="a")
                b_t = workp.tile([P, D], F32R, tag="b")
                # wposc on ACT (idle otherwise): Copy activation with
                # per-partition scale = (w*pos)_b
                nc.scalar.mul(wposc[:], c_b, wpos[:, b : b + 1])
                # B = c*pos_b - r  (no wposc dependency, issue first)
                nc.vector.scalar_tensor_tensor(
                    b_t[:], c_b, pk[:, b : b + 1], r_b, MUL, SUB
                )
                # A = r*w + wposc
                nc.vector.scalar_tensor_tensor(
                    a_t[:], r_b, pk[:, NB : NB + 1], wposc[:], MUL, ADD
                )

                ps = psump.tile([P, 2, D], F32, space="PSUM", tag="ps")
                pss.append(ps)
                # b_t-operand matmul first in each accumulation group: b_t
                # is ready before a_t (it doesn't depend on wposc), so PE
                # can start each batch ~0.4us earlier when its queue is free
                for m in range(2):
                    msl = slice(m * P, (m + 1) * P)
                    nc.tensor.matmul(
                        ps[:, m, :], b_t[:, msl], wposc[:], start=True, stop=False
                    )
                    nc.tensor.matmul(
                        ps[:, m, :], a_t[:, msl], r_b, start=False, stop=True
                    )
                if b >= LAG:
                    emit_drain(b - LAG)
            for b in range(NB - LAG, NB):
                emit_drain(b)
    nc.compile()
    return nc


_NC = None


def _get_nc():
    global _NC
    if _NC is None:
        _NC = build_bass()
    return _NC


def make_in_maps(questions, q_position, word_emb, cmp_emb, weighted_q):
    questions = np.asarray(questions)
    q_position = np.asarray(q_position, dtype=np.float32)
    word_emb = np.asarray(word_emb, dtype=np.float32)
    cmp_emb = np.asarray(cmp_emb, dtype=np.float32)
    weighted_q = np.asarray(weighted_q, dtype=np.float32)

    # interleaved table: row q = [word_emb[q] ++ cmp_emb[q]]  -> [V, 2D]
    tables = np.ascontiguousarray(
        np.concatenate([word_emb, cmp_emb], axis=1).astype(TBL_NP)
    )
    wq = weighted_q.reshape(S, 1)

    in_maps = []
    for core in range(NCORES):
        bs = slice(core * NB, (core + 1) * NB)
        pk = np.concatenate([q_position[bs].T, wq], axis=1).astype(np.float32)
        in_maps.append(
            {
                "tables": tables,
                "q2": np.ascontiguousarray(questions[bs].T.astype(np.int32)),
                "pk": np.ascontiguousarray(pk),
            }
        )
    return in_maps


def kernel(questions, q_position, word_emb, cmp_emb, weighted_q):
    global LAST_EXEC_NS, LAST_RESULTS
    in_maps = make_in_maps(questions, q_position, word_emb, cmp_emb, weighted_q)
    nc = _get_nc()
    res = run_bass_kernel_spmd(nc, in_maps, list(range(NCORES)), trace=TRACE)
    LAST_EXEC_NS = res.exec_time_ns
    LAST_RESULTS = res

    # [NCORES, NB, P, 2, D] -> C [B, 256, 256] with row d = m*128 + p
    outc = np.stack([res.results[c]["outc"] for c in range(NCORES)], axis=0)
    c_all = (
        outc.reshape(B, P, 2, D).transpose(0, 2, 1, 3).reshape(B, 2 * P, D)
    )
    ct = c_all.transpose(0, 2, 1)
    real = ((c_all + ct) * 0.5).astype(np.float32)
    imag = ((c_all - ct) * 0.5).astype(np.float32)
    return real, imag


# revision 23
# speedup vs baseline: 1.0229x; 1.0229x over previous
"""Trainium2 Bass kernel for nn_CMPModel (complex density matrix).

Math (per batch b, S=128 tokens, D=256):
    R = word_emb[questions[b]]                # [S, D]
    I = cmp_emb[questions[b]] * pos[b][:, None]
    real = R^T W R + I^T W I                  # symmetric   (W = diag(weighted_q))
    imag = I^T W R - R^T W I                  # antisymmetric

We compute only C = real + imag on device. Since diag weights can migrate
to either matmul operand (they depend only on the contraction index s),
two PSUM-accumulated products with 3 prepped operand tiles:
    C = A^T r + B^T wposc
      wposc = (w*pos)*c
      A     = w*r + wposc
      B     = pos*c - r
check (per element, sum over s):
    A^T r       = R^T W R + I^T W R
    B^T wposc   = sum_s (pos*c - r)*(w*pos*c) = I^T W I - R^T W I
and recover on host during unshard (exact by symmetry):
    real = (C + C^T)/2,  imag = (C - C^T)/2.

Sharding: data-parallel over batch, 8 batches per core, embedding tables
replicated. Gather via gpsimd indirect DMA against a host-interleaved
[V, 2D] table (row q = word_emb[q] ++ cmp_emb[q]), one [P,1]-offset
gather per batch. Multi-offset indirect DMA was probed on HW and is
broken (the ucode does not read the offset values from the q2 tile at
all; source addresses come out linear in the dest run index with a
constant sub-row misalignment), so one gather instruction per batch is
a hard floor: the Pool engine's serial descriptor generation
(~994ns fixed + 0.34ns/desc per instruction) paces the pipeline at
~1.4us/batch.

Engine balance per batch (each stage must stay under the ~1.4us gather
pace): ACT does wposc (activation Copy with per-partition scale) and
half the PSUM->SBUF copies; DVE does the two scalar_tensor_tensor preps
and the other half of the copies; PE does 4 LDWEIGHTS+MATMUL pairs
(bf16: ~420ns/MM with LDW overlapped). All compute rides inside the
gather window; the tail after the last gather is
data(+0.8) -> wposc -> a_t -> 4 MMs -> half-copies -> out-DMAs split
across the sync and ACT HWDGE sequencers (issue is ~0.65us serial per
instruction per engine).

Measured HW landmarks (fast-clock runs; DVFS adds ~10% run-to-run):
start rendezvous ~3.2us, TPBBaseLd ~1.2us, q2-on-sync done ~7.2us,
first gather 9.0us, gather pace 1.43us, last out-DMA issued 26.4us,
exec ~30.9us (vs 40.4us baseline). Alternatives probed and rejected on
HW: multi-offset indirect DMA (ucode broken, ignores offset values),
dma_gather ucode (2.7us/inst + 8us library load), pre-barrier q2 load
(holds the tile entry barrier), mid-pipeline ACT out-DMAs (blows ACT's
per-batch budget).
"""

import ml_dtypes
import numpy as np

import concourse.bacc as bacc
import concourse.bass as bass
import concourse.mybir as mybir
import concourse.tile as tile
from concourse.bass_utils import run_bass_kernel_spmd

V, D, S, B = 50000, 256, 128, 64
NCORES = 8
NB = B // NCORES          # batches per core
P = 128
F32 = mybir.dt.float32
# matmul input dtype. bf16 showed no speedup in earlier sessions because
# the span was gather-bound and PE hid underneath; after the pipeline
# fixes PE surfaced as the tail pacer (~1.7us/batch f32r: 4 LDW+MM pairs
# at ~240ns fixed overhead each), so bf16's faster weight loads + 2x DVE
# rate now matter. rel err ~3.5e-3 vs the 2e-2 gate.
USE_BF16 = True
F32R = mybir.dt.bfloat16 if USE_BF16 else mybir.dt.float32r
TBL_NP = ml_dtypes.bfloat16 if USE_BF16 else np.float32
MUL = mybir.AluOpType.mult
ADD = mybir.AluOpType.add
SUB = mybir.AluOpType.subtract

# set by test harness: trace the run and stash exec_time_ns
TRACE = False
LAST_EXEC_NS = None
LAST_RESULTS = None


def build_bass():
    nc = bacc.Bacc("TRN2", enable_partition_id=False)
    tables = nc.declare_dram_parameter("tables", [V, 2 * D], F32R, isOutput=False)
    q2_d = nc.declare_dram_parameter("q2", [P, NB], mybir.dt.int32, isOutput=False)
    # pos and wq packed into ONE upload: wpos (the first ACT op) then has a
    # single DMA-sem wait, so no EventSemaphore split lands before the
    # auto-inserted ACT_TABLE_LOAD — measured to pull the table load from
    # t=10.7us back to t=6.8us (it otherwise blocks every wposc).
    pk_d = nc.declare_dram_parameter("pk", [P, NB + 1], F32, isOutput=False)
    # output in bf16: the PSUM->SBUF copies convert f32->bf16, halving
    # out-DMA bytes (the final DMA transfer is on the critical tail);
    # adds ~4e-3 rounding on top of the 3.5e-3 compute error, still
    # well under the 2e-2 gate. Host unshards back to f32.
    out_d = nc.declare_dram_parameter("outc", [NB, P, 2, D], mybir.dt.bfloat16, isOutput=True)

    # (A pre-barrier q2 load via raw tensor + manual sem was tried and
    # measured WORSE by ~1us: gpsimd then holds the tile entry barrier
    # until q2 lands, so the barrier cost moves after the DMA instead of
    # overlapping it. Keep q2 as the first in-context gpsimd DMA.)
    with tile.TileContext(nc) as tc:
        with (
            tc.tile_pool(name="const", bufs=1) as constp,
            tc.tile_pool(name="gather", bufs=NB) as gatherp,
            tc.tile_pool(name="work", bufs=3 * NB) as workp,
            tc.tile_pool(name="outp", bufs=8) as outp,
            tc.tile_pool(name="psum", bufs=8, space="PSUM") as psump,
        ):
            # q2 on sync, FIRST: sync's post-barrier DMA dispatch starts
            # ~0.7us before gpsimd's would, and gpsimd then enters the
            # gather stream directly (its first instruction waits the q2
            # DMA sem). pk second on sync; wpos isn't needed until the
            # first gather's data lands, so its extra latency is free.
            q2 = constp.tile([P, NB], mybir.dt.int32)
            pk = constp.tile([P, NB + 1], F32)
            nc.sync.dma_start(out=q2[:], in_=q2_d[:])
            nc.sync.dma_start(out=pk[:], in_=pk_d[:])
            # wpos on ACT: ACT is otherwise idle here, and making this the
            # first activation hoists the ACT_TABLE_LOAD to the program
            # start (it otherwise lands behind the first gather's sem wait,
            # delaying every wposc).
            wpos = constp.tile([P, NB], F32)
            nc.scalar.mul(wpos[:], pk[:, 0:NB], pk[:, NB : NB + 1])

            # all gathers up front: gpsimd desc-gen is the serial resource;
            # per-batch tiles with NB bufs so the stream never stalls on slots
            rcs = []
            for b in range(NB):
                rc = gatherp.tile([P, 2 * D], F32R, tag=f"rc")
                rcs.append(rc)
                nc.gpsimd.indirect_dma_start(
                    out=rc[:],
                    out_offset=None,
                    in_=tables[:],
                    in_offset=bass.IndirectOffsetOnAxis(ap=q2[:, b : b + 1], axis=0),
                )

            # per-batch pipeline: each batch's prep only needs its own gather,
            # so compute trails the gather stream by ~1 batch.
            # PSUM evacuation is interleaved with a 2-batch lag: emitting all
            # copies after all preps (tried) serializes every copy behind the
            # LAST prep in the in-order engine streams — batch 0's copy then
            # runs at T+26us with data ready at T+19us, and the out-DMA queue
            # backs up at the tail. With lag 2, copy b-2 waits on matmuls
            # that are already done when prep b is emitted, so neither stream
            # stalls. Even batches copy on DVE, odd on ACT; out-DMA issue
            # (~0.65us serial per instruction) all on the sync sequencer,
            # spread across the pipeline instead of bunched at the end.
            LAG = 2

            def emit_drain(b):
                # Tail DMAs (b5, b6, last half) issue on ACT: out-DMA issue
                # is ~0.65us serial per instruction, and sync alone bunches
                # the last 5 issues after the copies. ACT is done with
                # wposc/copies by then; mid-pipeline ACT DMAs (tried) blow
                # its per-batch budget and delay the wposc stream.
                out_sb = outp.tile([P, 2, D], mybir.dt.bfloat16, tag="osb")
                if b < NB - 1:
                    if b % 2 == 0:
                        nc.vector.tensor_copy(out_sb[:], pss[b][:])
                    else:
                        nc.scalar.copy(out_sb[:], pss[b][:])
                    # ACT only drains b5 and the final half-tile: giving it
                    # b6's DMA too (tried) queues the last half-copy behind
                    # that issue and delays the final DMA by ~0.7us
                    eng = nc.scalar if b == NB - 3 else nc.sync
                    eng.dma_start(out=out_d[b], in_=out_sb[:])
                else:
                    # last batch drains in halves so its DMA overlaps its copy
                    nc.vector.tensor_copy(out_sb[:, 0, :], pss[b][:, 0, :])
                    nc.sync.dma_start(out=out_d[b, :, 0], in_=out_sb[:, 0, :])
                    nc.scalar.copy(out_sb[:, 1, :], pss[b][:, 1, :])
                    nc.scalar.dma_start(out=out_d[b, :, 1], in_=out_sb[:, 1, :])

            pss = []
            wposcs = []
            for b in range(NB):
                rc = rcs[b]
                r_b = rc[:, 0:D]
                c_b = rc[:, D : 2 * D]
                wposc = workp.tile([P, D], F32R, tag="wposc")
                wposcs.append(wposc)
                a_t = workp.tile([P, D], F32R, tag="a")
                b_t = workp.tile([P, D], F32R, tag="b")
                # wposc on ACT (idle otherwise): Copy activation with
                # per-partition scale = (w*pos)_b
                nc.scalar.mul(wposc[:], c_b, wpos[:, b : b + 1])
                # B = c*pos_b - r  (no wposc dependency, issue first)
                nc.vector.scalar_tensor_tensor(
                    b_t[:], c_b, pk[:, b : b + 1], r_b, MUL, SUB
                )
                # A = r*w + wposc
                nc.vector.scalar_tensor_tensor(
                    a_t[:], r_b, pk[:, NB : NB + 1], wposc[:], MUL, ADD
                )

                ps = psump.tile([P, 2, D], F32, space="PSUM", tag="ps")
                pss.append(ps)
                # b_t-operand matmul first in each accumulation group: b_t
                # is ready before a_t (it doesn't depend on wposc), so PE
                # can start each batch ~0.4us earlier when its queue is free
                for m in range(2):
                    msl = slice(m * P, (m + 1) * P)
                    nc.tensor.matmul(
                        ps[:, m, :], b_t[:, msl], wposc[:], start=True, stop=False
                    )
                    nc.tensor.matmul(
                        ps[:, m, :], a_t[:, msl], r_b, start=False, stop=True
                    )
                if b >= LAG:
                    emit_drain(b - LAG)
            for b in range(NB - LAG, NB):
                emit_drain(b)
    nc.compile()
    return nc


_NC = None


def _get_nc():
    global _NC
    if _NC is None:
        _NC = build_bass()
    return _NC


def make_in_maps(questions, q_position, word_emb, cmp_emb, weighted_q):
    questions = np.asarray(questions)
    q_position = np.asarray(q_position, dtype=np.float32)
    word_emb = np.asarray(word_emb, dtype=np.float32)
    cmp_emb = np.asarray(cmp_emb, dtype=np.float32)
    weighted_q = np.asarray(weighted_q, dtype=np.float32)

    # interleaved table: row q = [word_emb[q] ++ cmp_emb[q]]  -> [V, 2D]
    tables = np.ascontiguousarray(
        np.concatenate([word_emb, cmp_emb], axis=1).astype(TBL_NP)
    )
    wq = weighted_q.reshape(S, 1)

    in_maps = []
    for core in range(NCORES):
        bs = slice(core * NB, (core + 1) * NB)
        pk = np.concatenate([q_position[bs].T, wq], axis=1).astype(np.float32)
        in_maps.append(
            {
                "tables": tables,
                "q2": np.ascontiguousarray(questions[bs].T.astype(np.int32)),
                "pk": np.ascontiguousarray(pk),
            }
        )
    return in_maps


def kernel(questions, q_position, word_emb, cmp_emb, weighted_q):
    global LAST_EXEC_NS, LAST_RESULTS
    in_maps = make_in_maps(questions, q_position, word_emb, cmp_emb, weighted_q)
    nc = _get_nc()
    res = run_bass_kernel_spmd(nc, in_maps, list(range(NCORES)), trace=TRACE)
    LAST_EXEC_NS = res.exec_time_ns
    LAST_RESULTS = res

    # [NCORES, NB, P, 2, D] -> C [B, 256, 256] with row d = m*128 + p
    outc = np.stack(
        [res.results[c]["outc"].astype(np.float32) for c in range(NCORES)], axis=0
    )
    c_all = (
        outc.reshape(B, P, 2, D).transpose(0, 2, 1, 3).reshape(B, 2 * P, D)
    )
    ct = c_all.transpose(0, 2, 1)
    real = ((c_all + ct) * 0.5).astype(np.float32)
    imag = ((c_all - ct) * 0.5).astype(np.float32)
    return real, imag
